# revision 29
# baseline (speedup 1.0000x reference)
"""Trainium2 Bass kernel for nn_Detector (YOLO-style detector decode).

Contract: kernel(**inputs) takes the FULL unsharded inputs from
setup_inputs() and returns the FULL [340704, 90] fp32 output. Internally
the batch dim (32) is sharded across 8 NeuronCores (4 images per core);
each core decodes its slice of all three scales and the host reassembles
the rows.

Design:
  - bf16 data path: inputs converted to bf16 on the host (halves load
    traffic), SBUF row tiles and DRAM outputs bf16 (halves store traffic),
    host upconverts to fp32. Elementwise rel err ~2^-9, far under the
    2e-2 gate.
  - The objectness channel travels as a HOST-PRE-TRANSPOSED fp32 plane
    ([128, 90] per image: partition = hw%128, col = (scale,chunk,anchor)),
    so sigmoid(p) > thresh is fp32-exact and the row mask never flips.
  - dw,dh also travel host-pre-transposed (bf16), so exp+sqrt for the box
    diagonal run ONCE over all images in a prologue: exactly 3 ScalarE
    table eras for the whole kernel (exp -> sqrt -> sigmoid).
  - PE transposes in bf16 (1-pass); PSUM tiles bf16, bufs=4.
  - All loads ride one HWDGE ring (nc.sync) in strict priority order
    (consts -> img0-small -> per-image 52-section-first); stores ride
    SWDGE (nc.gpsimd) so they never delay the load stream. 10 load DMAs
    with sem-lane-reuse-safe ordering (8 lanes exist).
  - Device rows use a COLUMN-GROUPED order [box 0:6 | point 6:18 |
    seg-coord 18:42 | seg-sig 42:90]; the big mask/scale passes are
    single unit-stride instructions whose broadcast operands come from
    PAIR-DUPLICATED tiles ([128, g, 2] with the value repeated). Host
    un-permutes columns during the gather, re-applies the row mask (the
    device ships the exact fp32-computed 0/1 mask in col 0) and fills
    in the n column.
  - DRAM outputs PARTITION-MAJOR [b, p, chunk, anchor, 90]: each
    partition line is one contiguous run; host un-permutes in the same
    pass.
"""
import numpy as np
import ml_dtypes

f32np = np.float32
bf16np = ml_dtypes.bfloat16

B = 32
N_CORES = 8
B_LOCAL = B // N_CORES

# (name, W, t, HW)
SCALES = [("52", 52, 8.0, 2704), ("26", 26, 16.0, 676), ("13", 13, 32.0, 169)]
CHUNKS = {name: (HW + 127) // 128 for name, _, _, HW in SCALES}  # 22, 6, 2
OFFS = {"52": 0, "26": 3 * 2704, "13": 3 * 2704 + 3 * 676}  # in xin free dim
XIN_F = 3 * (2704 + 676 + 169)  # 10647
GS = {name: 3 * CHUNKS[name] for name, _, _, HW in SCALES}  # 66, 18, 6
GOFF = {"52": 0, "26": 66, "13": 84}
GTOT = 90

# device -> reference column permutation (COLPERM[i] = device col of ref col i)
COLPERM = list(range(18))
for _i in range(24):
    COLPERM += [18 + _i, 42 + 2 * _i, 43 + 2 * _i]

# fp32 consts column layout: [128, NCF]
_THR = 0
_NTAB = 1                              # 4 cols: n value per local batch
NCF = 5
# bf16 consts column layout: [128, NCB]
_AWV = 0                               # 2*GTOT cols: (aw,ah) per (scale,c,a)
_IXY = {"52": 180, "26": 224, "13": 236}  # 2T cols each: (ix,iy) per chunk
NCB = 240

_CACHE = {}


def _build_nc():
    import concourse.bacc as bacc
    import concourse.tile as tile
    from concourse import mybir
    from concourse.masks import make_identity
    from concourse.tile_rust import add_dep_helper

    f32 = mybir.dt.float32
    bf16 = mybir.dt.bfloat16
    AF = mybir.ActivationFunctionType
    OP = mybir.AluOpType

    nc = bacc.Bacc("TRN2", target_bir_lowering=False, debug=False)
    xin = nc.declare_dram_parameter("xin", [B_LOCAL, 90, XIN_F], bf16,
                                    isOutput=False)
    # merged const+plane tensors: one DMA each (only 8 HWDGE sem lanes
    # exist, and a ring holds only ~2 big loads of descriptors)
    cfp = nc.declare_dram_parameter("cfp", [128, NCF + B_LOCAL * GTOT], f32,
                                    isOutput=False)
    cwh = nc.declare_dram_parameter(
        "cwh", [128, NCB + B_LOCAL * GTOT * 2], bf16, isOutput=False)
    ys = {}
    for name, _, _, HW in SCALES:
        # partition-major: [b, p, chunk, anchor, 90]; host un-permutes
        ys[name] = nc.declare_dram_parameter(
            f"y{name}", [B_LOCAL, 128, GS[name] * 90], bf16, isOutput=True)

    with tile.TileContext(nc) as tc:
        with (
            tc.tile_pool(name="single", bufs=1) as single,
            tc.tile_pool(name="inp", bufs=4) as in_pool,
            tc.tile_pool(name="outp", bufs=6) as out_pool,
            tc.tile_pool(name="psum", bufs=4, space="PSUM") as psum_pool,
        ):
            ident = single.tile([128, 128], bf16)
            make_identity(nc, ident[:])

            # ---- loads: 10 HWDGE DMAs on one ring, in strict priority
            # order. Only 8 sem lanes exist (round-robin), so DMAs #9/#10
            # reuse lanes #1/#2 — therefore lanes 1-2 must belong to
            # EARLY-consumed DMAs (cfp: prologue-only; img0-small:
            # transposed by ~20us). cwh (read by evacuations all kernel)
            # sits at #3 so its lane is never reused. ----
            in_ts = []
            for _b in range(B_LOCAL):
                in_t = in_pool.tile([90, XIN_F], bf16, tag="intile")
                in_ts.append(in_t)
            O52 = 3 * 2704
            SPL = 12 * 128  # img0 52-section split: first 12 chunks
            cfp_t = single.tile([128, NCF + B_LOCAL * GTOT], f32)
            nc.sync.dma_start(out=cfp_t[:], in_=cfp[:])
            ctf = cfp_t[:, 0:NCF]
            pt = cfp_t[:, NCF:]
            # img0 small scales first: its 26/13 transposes start earliest
            nc.sync.dma_start(out=in_ts[0][:, O52:], in_=xin[0][:, O52:])
            cwh_t = single.tile([128, NCB + B_LOCAL * GTOT * 2], bf16)
            nc.sync.dma_start(out=cwh_t[:], in_=cwh[:])
            ctb = cwh_t[:, 0:NCB]
            wh_t = cwh_t[:, NCB:].rearrange("p (g k) -> p g k", k=2)
            # img0's 52 section in two pieces (per-anchor strided ranges)
            v0o = in_ts[0][:, 0:O52].rearrange("p (a hw) -> p a hw", a=3)
            v0i = xin[0][:, 0:O52].rearrange("p (a hw) -> p a hw", a=3)
            nc.sync.dma_start(out=v0o[:, :, 0:SPL], in_=v0i[:, :, 0:SPL])
            nc.sync.dma_start(out=v0o[:, :, SPL:], in_=v0i[:, :, SPL:])
            for b in range(1, B_LOCAL):
                if b < B_LOCAL - 1:
                    # 52 section first: transposes start at ~60% loaded
                    nc.sync.dma_start(out=in_ts[b][:, 0:O52],
                                      in_=xin[b][:, 0:O52])
                    nc.sync.dma_start(out=in_ts[b][:, O52:],
                                      in_=xin[b][:, O52:])
                else:
                    nc.sync.dma_start(out=in_ts[b][:], in_=xin[b][:])

            # ---------- prologue A (sigmoid era, needs only cfp):
            # obj sigmoid + mask + the [mask, sig] column pair. img0's
            # small-scale evacuations (also sigmoid era) start right
            # after, ~7us before the exp/sqrt chain would allow ----
            sig_t = single.tile([128, B_LOCAL * GTOT], f32)
            nc.scalar.activation(sig_t[:], pt, AF.Sigmoid)
            mask_t = single.tile([128, B_LOCAL * GTOT], f32)
            nc.vector.tensor_scalar(mask_t[:], sig_t[:],
                                    ctf[:, _THR:_THR + 1], None, op0=OP.is_gt)
            # cols 0:2 = [mask, sig_raw]; the HOST re-applies the row mask
            # and writes n*mask into col 0 during the gather (it knows n)
            nm01 = single.tile([128, B_LOCAL * GTOT, 2], bf16)
            nc.vector.tensor_copy(nm01[:, :, 0], mask_t[:])
            nc.vector.tensor_copy(nm01[:, :, 1], sig_t[:])

            prev_fin = None
            last_fin = None
            sm2 = None
            fin_q = []
            SCJOB = {n: (W_, t_, HW_) for n, W_, t_, HW_ in SCALES}
            work = [(0, "26"), (0, "13"), None, (0, "52")] + [
                (b_, n_) for b_ in range(1, B_LOCAL)
                for n_ in ("52", "26", "13")]
            for job in work:
                if job is None:
                    # ---------- prologue B: exp -> sqrt eras ----------
                    nc.scalar.activation(wh_t, wh_t, AF.Exp)
                    nc.vector.tensor_mul(
                        wh_t.rearrange("p (b g) k -> p b g k", b=B_LOCAL),
                        wh_t.rearrange("p (b g) k -> p b g k", b=B_LOCAL),
                        ctb[:, _AWV:_AWV + 2 * GTOT].rearrange(
                            "p (g k) -> p g k", k=2).unsqueeze(1)
                        .broadcast_to((128, B_LOCAL, GTOT, 2)))
                    sq_t = single.tile([128, B_LOCAL * GTOT, 2], bf16)
                    nc.vector.tensor_mul(sq_t[:], wh_t, wh_t)
                    q_t = single.tile([128, B_LOCAL * GTOT], bf16)
                    nc.vector.tensor_add(q_t[:], sq_t[:, :, 0], sq_t[:, :, 1])
                    s_t = single.tile([128, B_LOCAL * GTOT], bf16)
                    nc.scalar.activation(s_t[:], q_t[:], AF.Sqrt,
                                         scale=1.0 / (416.0 * 416.0))
                    # pair-duplicated broadcast source for the s pass
                    sm2 = single.tile([128, B_LOCAL * GTOT, 2], bf16)
                    nc.vector.tensor_copy(sm2[:, :, 0], s_t[:])
                    nc.vector.tensor_copy(sm2[:, :, 1], s_t[:])
                else:
                    b, name = job
                    W, t, HW = SCJOB[name]
                    in_t = in_ts[b]
                    T = CHUNKS[name]
                    G = GS[name]
                    go = b * GTOT + GOFF[name]
                    vs = in_t[:, OFFS[name]:OFFS[name] + 3 * HW].rearrange(
                        "p (a hw) -> p a hw", a=3)
                    out_t = out_pool.tile([128, G, 90], bf16, tag="outtile")
                    og = out_t[:]
                    ixyo = _IXY[name]

                    # -- transpose + PSUM evacuation (sigmoid era) --
                    for c0 in range(0, T, 4):
                        nch = min(4, T - c0)
                        gcg = nch * 3
                        ps = psum_pool.tile([128, 12, 128], bf16, tag="ps")
                        for ci in range(nch):
                            c = c0 + ci
                            w = min(128, HW - c * 128)
                            for a in range(3):
                                nc.tensor.transpose(
                                    ps[:w, ci * 3 + a, 0:90],
                                    vs[:, a, c * 128:c * 128 + w],
                                    ident[0:90, 0:90])
                        pg = ps[:, 0:gcg, :]
                        osl = og[:, c0 * 3:c0 * 3 + gcg, :]
                        # seg sigmoids -> contiguous cols 42:90
                        nc.scalar.activation(
                            osl[:, :, 42:90].rearrange(
                                "p g (i j) -> p g i j", j=2),
                            pg[:, :, 18:90].rearrange(
                                "p g (i j) -> p g i j", j=3)[:, :, :, 1:3],
                            AF.Sigmoid)
                        # dx,dy + grid -> cols 2:4 (fused from PSUM)
                        _ixy_ins = nc.vector.tensor_add(
                            osl.rearrange("p (c a) k -> p c a k", a=3)[
                                :, :, :, 2:4],
                            pg.rearrange("p (c a) k -> p c a k", a=3)[
                                :, :, :, 1:3],
                            ctb[:, ixyo + 2 * c0:ixyo + 2 * (c0 + nch)]
                            .rearrange("p (c k) -> p c k", k=2).unsqueeze(2)
                            .broadcast_to((128, nch, 3, 2)))
                        if prev_fin is not None:
                            # schedule hint: finish image b-1's finishers
                            # before image b's DVE evac work, so stores
                            # flow early instead of piling into a
                            # terminal drain
                            add_dep_helper(_ixy_ins.ins, prev_fin.ins,
                                           sync=True,
                                           reason="drain stores early")
                            prev_fin = None
                        # raw point block -> cols 6:18 (alternate engines
                        # to balance ACT/DVE in the evacuation path)
                        if (c0 // 4) % 2 == 0:
                            nc.scalar.copy(osl[:, :, 6:18], pg[:, :, 6:18])
                        else:
                            nc.vector.tensor_copy(osl[:, :, 6:18],
                                                  pg[:, :, 6:18])
                        # raw seg coords -> contiguous cols 18:42
                        nc.vector.tensor_copy(osl[:, :, 18:42],
                                              pg[:, :, 18:90:3])
                    fin_q.append((b, name, t, go, G, out_t))

                # finishers run once sm2 exists (deferred for the two
                # pre-prologue scales)
                if sm2 is None:
                    continue
                for fb, fname, ft, fgo, fG, fout_t in fin_q:
                    fog = fout_t[:]
                    halves = ([(0, fG // 2), (fG // 2, fG)]
                              if fname == "52" else [(0, fG)])
                    for g0, g1 in halves:
                        gl = g1 - g0
                        ogh = fog[:, g0:g1, :]
                        gh = fgo + g0
                        # [mask, sig_raw] pair from prologue
                        nc.vector.tensor_copy(
                            ogh[:, :, 0:2], nm01[:, gh:gh + gl, :])
                        # w,h raw copy from prologue
                        nc.vector.tensor_copy(ogh[:, :, 4:6],
                                              wh_t[:, gh:gh + gl, :])
                        # cx,cy: * t via ACT input-scale (offloads DVE)
                        nc.scalar.activation(
                            ogh[:, :, 2:4], ogh[:, :, 2:4],
                            AF.Copy, scale=float(ft))
                        # point+seg coords * s (mask applied by the host)
                        last_fin = nc.vector.tensor_mul(
                            ogh[:, :, 6:42].rearrange(
                                "p g (i j) -> p g i j", j=2),
                            ogh[:, :, 6:42].rearrange(
                                "p g (i j) -> p g i j", j=2),
                            sm2[:, gh:gh + gl, :].unsqueeze(2).broadcast_to(
                                (128, gl, 18, 2)))
                        nc.gpsimd.dma_start(
                            out=ys[fname][fb][:, g0 * 90:g1 * 90],
                            in_=ogh.rearrange("p g k -> p (g k)"))
                    if fname == ("52" if fb == 0 else "13"):
                        prev_fin = last_fin
                fin_q = []
    nc.compile()
    return nc


def _host_consts(core, anchors, thresh):
    ctf = np.zeros((128, NCF), f32np)
    ctf[:, _THR] = f32np(thresh[0])
    for b in range(B_LOCAL):
        ctf[:, _NTAB + b] = f32np(core * B_LOCAL + b)
    ctb = np.zeros((128, NCB), bf16np)
    # anchor vector per og column group g=(scale,chunk,anchor), interleaved
    awv = np.zeros((GTOT, 2), f32np)
    for name, W, t, HW in SCALES:
        a = anchors[name].astype(f32np)  # [3, 2]
        go = GOFF[name]
        T = CHUNKS[name]
        awv[go:go + GS[name]] = np.tile(a, (T, 1))
        hw = np.arange(T)[None, :] * 128 + np.arange(128)[:, None]  # [128, T]
        o = _IXY[name]
        ctb[:, o:o + 2 * T:2] = (hw % W).astype(bf16np)
        ctb[:, o + 1:o + 2 * T:2] = (hw // W).astype(bf16np)
    ctb[:, _AWV:_AWV + 2 * GTOT] = awv.reshape(-1)[None, :].astype(bf16np)
    return ctf, ctb


def _f32_to_bf16(x):
    """Round-to-nearest-even fp32 -> bf16, much faster than ml_dtypes astype."""
    u = np.ascontiguousarray(x, f32np).view(np.uint32)
    r = ((u + np.uint32(0x7FFF) + ((u >> np.uint32(16)) & np.uint32(1)))
         >> np.uint32(16)).astype(np.uint16)
    return r.view(bf16np)


def _bf16_to_f32(x):
    u = np.asarray(x).view(np.uint16).astype(np.uint32) << np.uint32(16)
    return u.view(f32np)


def _make_in_maps(out13, out26, out52, anchors, thresh):
    xs_full = {
        "13": np.asarray(out13, f32np).reshape(B, 3, 90, 169),
        "26": np.asarray(out26, f32np).reshape(B, 3, 90, 676),
        "52": np.asarray(out52, f32np).reshape(B, 3, 90, 2704),
    }
    # pack per image: [90, concat over scales of (a, hw)] in bf16
    xin = np.empty((B, 90, XIN_F), bf16np)
    # host-pre-transposed planes: pin (p, fp32), whin (dw,dh, bf16)
    pin = np.zeros((B, 128, GTOT), f32np)
    whin = np.zeros((B, 128, GTOT, 2), bf16np)
    for name, _, _, HW in SCALES:
        o = OFFS[name]
        xin[:, :, o:o + 3 * HW] = _f32_to_bf16(xs_full[name].transpose(
            0, 2, 1, 3)).reshape(B, 90, 3 * HW)
        T = CHUNKS[name]
        go = GOFF[name]
        p = np.zeros((B, 3, T * 128), f32np)
        p[:, :, :HW] = xs_full[name][:, :, 0, :]
        pin[:, :, go:go + GS[name]] = p.reshape(B, 3, T, 128).transpose(
            0, 3, 2, 1).reshape(B, 128, T * 3)
        wh = np.zeros((B, 3, 2, T * 128), f32np)
        wh[:, :, :, :HW] = xs_full[name][:, :, 3:5, :]
        whin[:, :, go:go + GS[name], :] = _f32_to_bf16(
            wh.reshape(B, 3, 2, T, 128).transpose(0, 4, 3, 1, 2)).reshape(
                B, 128, T * 3, 2)
    in_maps = []
    for core in range(N_CORES):
        bs = slice(core * B_LOCAL, (core + 1) * B_LOCAL)
        ctf, ctb = _host_consts(core, anchors, thresh)
        cfp = np.concatenate(
            [ctf, pin[bs].transpose(1, 0, 2).reshape(128, B_LOCAL * GTOT)],
            axis=1)
        cwh = np.concatenate(
            [ctb, whin[bs].transpose(1, 0, 2, 3).reshape(
                128, B_LOCAL * GTOT * 2)], axis=1)
        m = {"xin": np.ascontiguousarray(xin[bs]),
             "cfp": np.ascontiguousarray(cfp),
             "cwh": np.ascontiguousarray(cwh)}
        in_maps.append(m)
    return in_maps


def kernel(out13, out26, out52, anchors13, anchors26, anchors52, thresh,
           case, **kw):
    from concourse.bass_utils import run_bass_kernel_spmd

    anchors = {"13": np.asarray(anchors13), "26": np.asarray(anchors26),
               "52": np.asarray(anchors52)}
    thresh = np.asarray(thresh, f32np)

    if "nc" not in _CACHE:
        _CACHE["nc"] = _build_nc()
    nc = _CACHE["nc"]

    in_maps = _make_in_maps(out13, out26, out52, anchors, thresh)
    res = run_bass_kernel_spmd(nc, in_maps, list(range(N_CORES))).results

    rows = {name: B * HW * 3 for name, _, _, HW in SCALES}
    out = np.empty((rows["13"] + rows["26"] + rows["52"], 90), f32np)
    region = {"13": 0, "26": rows["13"], "52": rows["13"] + rows["26"]}
    colperm = np.asarray(COLPERM)
    for core in range(N_CORES):
        r = res[core]
        for name, _, _, HW in SCALES:
            T = CHUNKS[name]
            # [B_LOCAL, 128, T, 3, 90] -> [B_LOCAL, T, 128, 3, 90] -> rows,
            # un-permuting columns and upconverting in the same pass; the
            # device ships raw rows with the exact 0/1 mask in col 0 — the
            # host applies the row mask and fills in n = image index
            arr = _bf16_to_f32(np.asarray(r[f"y{name}"])).reshape(
                B_LOCAL, 128, T, 3, 90).transpose(0, 2, 1, 3, 4)
            arr = arr[..., colperm].reshape(
                B_LOCAL, T * 128, 3, 90)[:, :HW]
            m = arr[..., 0:1]
            arr = arr * m
            nvec = (core * B_LOCAL + np.arange(B_LOCAL)).astype(f32np)
            arr[..., 0] = m[..., 0] * nvec[:, None, None]
            n = B_LOCAL * HW * 3
            out[region[name] + core * n:region[name] + (core + 1) * n] = \
                arr.reshape(n, 90)
    return out


# revision 31
# speedup vs baseline: 1.0014x; 1.0014x over previous
"""Trainium2 Bass kernel for nn_Detector (YOLO-style detector decode).

Contract: kernel(**inputs) takes the FULL unsharded inputs from
setup_inputs() and returns the FULL [340704, 90] fp32 output. Internally
the batch dim (32) is sharded across 8 NeuronCores (4 images per core);
each core decodes its slice of all three scales and the host reassembles
the rows.

Design:
  - bf16 data path: inputs converted to bf16 on the host (halves load
    traffic), SBUF row tiles and DRAM outputs bf16 (halves store traffic),
    host upconverts to fp32. Elementwise rel err ~2^-9, far under the
    2e-2 gate.
  - The objectness channel travels as a HOST-PRE-TRANSPOSED fp32 plane
    ([128, 90] per image: partition = hw%128, col = (scale,chunk,anchor)),
    so sigmoid(p) > thresh is fp32-exact and the row mask never flips.
  - dw,dh also travel host-pre-transposed (bf16), so exp+sqrt for the box
    diagonal run ONCE over all images in a prologue: exactly 3 ScalarE
    table eras for the whole kernel (exp -> sqrt -> sigmoid).
  - PE transposes in bf16 (1-pass); PSUM tiles bf16, bufs=4.
  - All loads ride one HWDGE ring (nc.sync) in strict priority order
    (consts -> img0-small -> per-image 52-section-first); stores ride
    SWDGE (nc.gpsimd) so they never delay the load stream. 10 load DMAs
    with sem-lane-reuse-safe ordering (8 lanes exist).
  - Device rows use a COLUMN-GROUPED order [box 0:6 | point 6:18 |
    seg-coord 18:42 | seg-sig 42:90]; the big mask/scale passes are
    single unit-stride instructions whose broadcast operands come from
    PAIR-DUPLICATED tiles ([128, g, 2] with the value repeated). Host
    un-permutes columns during the gather, re-applies the row mask (the
    device ships the exact fp32-computed 0/1 mask in col 0) and fills
    in the n column.
  - DRAM outputs PARTITION-MAJOR [b, p, chunk, anchor, 90]: each
    partition line is one contiguous run; host un-permutes in the same
    pass.
"""
import numpy as np
import ml_dtypes

f32np = np.float32
bf16np = ml_dtypes.bfloat16

B = 32
N_CORES = 8
B_LOCAL = B // N_CORES

# (name, W, t, HW)
SCALES = [("52", 52, 8.0, 2704), ("26", 26, 16.0, 676), ("13", 13, 32.0, 169)]
CHUNKS = {name: (HW + 127) // 128 for name, _, _, HW in SCALES}  # 22, 6, 2
OFFS = {"52": 0, "26": 3 * 2704, "13": 3 * 2704 + 3 * 676}  # in xin free dim
XIN_F = 3 * (2704 + 676 + 169)  # 10647
GS = {name: 3 * CHUNKS[name] for name, _, _, HW in SCALES}  # 66, 18, 6
GOFF = {"52": 0, "26": 66, "13": 84}
GTOT = 90

# device -> reference column permutation (COLPERM[i] = device col of ref col i)
COLPERM = list(range(18))
for _i in range(24):
    COLPERM += [18 + _i, 42 + 2 * _i, 43 + 2 * _i]

# fp32 consts column layout: [128, NCF]
_THR = 0
_NTAB = 1                              # 4 cols: n value per local batch
NCF = 5
# bf16 consts column layout: [128, NCB]
_AWV = 0                               # 2*GTOT cols: (aw,ah) per (scale,c,a)
_IXY = {"52": 180, "26": 224, "13": 236}  # 2T cols each: (ix,iy) per chunk
NCB = 240

_CACHE = {}


def _build_nc():
    import concourse.bacc as bacc
    import concourse.tile as tile
    from concourse import mybir
    from concourse.masks import make_identity
    from concourse.tile_rust import add_dep_helper

    f32 = mybir.dt.float32
    bf16 = mybir.dt.bfloat16
    AF = mybir.ActivationFunctionType
    OP = mybir.AluOpType

    nc = bacc.Bacc("TRN2", target_bir_lowering=False, debug=False)
    xin = nc.declare_dram_parameter("xin", [B_LOCAL, 90, XIN_F], bf16,
                                    isOutput=False)
    # merged const+plane tensors: one DMA each (only 8 HWDGE sem lanes
    # exist, and a ring holds only ~2 big loads of descriptors)
    cfp = nc.declare_dram_parameter("cfp", [128, NCF + B_LOCAL * GTOT], f32,
                                    isOutput=False)
    cwh = nc.declare_dram_parameter(
        "cwh", [128, NCB + B_LOCAL * GTOT * 2], bf16, isOutput=False)
    ys = {}
    for name, _, _, HW in SCALES:
        # partition-major: [b, p, chunk, anchor, 90]; host un-permutes
        ys[name] = nc.declare_dram_parameter(
            f"y{name}", [B_LOCAL, 128, GS[name] * 90], bf16, isOutput=True)

    with tile.TileContext(nc) as tc:
        with (
            tc.tile_pool(name="single", bufs=1) as single,
            tc.tile_pool(name="inp", bufs=4) as in_pool,
            tc.tile_pool(name="outp", bufs=6) as out_pool,
            tc.tile_pool(name="psum", bufs=4, space="PSUM") as psum_pool,
        ):
            ident = single.tile([128, 128], bf16)
            make_identity(nc, ident[:])

            # ---- loads: 10 HWDGE DMAs on one ring, in strict priority
            # order. Only 8 sem lanes exist (round-robin), so DMAs #9/#10
            # reuse lanes #1/#2 — therefore lanes 1-2 must belong to
            # EARLY-consumed DMAs (cfp: prologue-only; img0-small:
            # transposed by ~20us). cwh (read by evacuations all kernel)
            # sits at #3 so its lane is never reused. ----
            in_ts = []
            for _b in range(B_LOCAL):
                in_t = in_pool.tile([90, XIN_F], bf16, tag="intile")
                in_ts.append(in_t)
            O52 = 3 * 2704
            SPL = 12 * 128  # img0 52-section split: first 12 chunks
            cfp_t = single.tile([128, NCF + B_LOCAL * GTOT], f32)
            nc.sync.dma_start(out=cfp_t[:], in_=cfp[:])
            ctf = cfp_t[:, 0:NCF]
            pt = cfp_t[:, NCF:]
            # img0 small scales first: its 26/13 transposes start earliest
            nc.sync.dma_start(out=in_ts[0][:, O52:], in_=xin[0][:, O52:])
            cwh_t = single.tile([128, NCB + B_LOCAL * GTOT * 2], bf16)
            nc.sync.dma_start(out=cwh_t[:], in_=cwh[:])
            ctb = cwh_t[:, 0:NCB]
            wh_t = cwh_t[:, NCB:].rearrange("p (g k) -> p g k", k=2)
            # img0's 52 section in two pieces (per-anchor strided ranges)
            v0o = in_ts[0][:, 0:O52].rearrange("p (a hw) -> p a hw", a=3)
            v0i = xin[0][:, 0:O52].rearrange("p (a hw) -> p a hw", a=3)
            nc.sync.dma_start(out=v0o[:, :, 0:SPL], in_=v0i[:, :, 0:SPL])
            nc.sync.dma_start(out=v0o[:, :, SPL:], in_=v0i[:, :, SPL:])
            for b in range(1, B_LOCAL):
                if b < B_LOCAL - 1:
                    # 52 section first: transposes start at ~60% loaded
                    nc.sync.dma_start(out=in_ts[b][:, 0:O52],
                                      in_=xin[b][:, 0:O52])
                    nc.sync.dma_start(out=in_ts[b][:, O52:],
                                      in_=xin[b][:, O52:])
                else:
                    nc.sync.dma_start(out=in_ts[b][:], in_=xin[b][:])

            # ---------- prologue A (sigmoid era, needs only cfp):
            # obj sigmoid + mask + the [mask, sig] column pair. img0's
            # small-scale evacuations (also sigmoid era) start right
            # after, ~7us before the exp/sqrt chain would allow ----
            sig_t = single.tile([128, B_LOCAL * GTOT], f32)
            nc.scalar.activation(sig_t[:], pt, AF.Sigmoid)
            mask_t = single.tile([128, B_LOCAL * GTOT], f32)
            nc.vector.tensor_scalar(mask_t[:], sig_t[:],
                                    ctf[:, _THR:_THR + 1], None, op0=OP.is_gt)
            # cols 0:2 = [mask, sig_raw]; the HOST re-applies the row mask
            # and writes n*mask into col 0 during the gather (it knows n)
            nm01 = single.tile([128, B_LOCAL * GTOT, 2], bf16)
            nc.vector.tensor_copy(nm01[:, :, 0], mask_t[:])
            nc.vector.tensor_copy(nm01[:, :, 1], sig_t[:])

            prev_fin = None
            last_fin = None
            sm2 = None
            fin_q = []
            SCJOB = {n: (W_, t_, HW_) for n, W_, t_, HW_ in SCALES}
            work = [(0, "26"), (0, "13"), None, (0, "52")] + [
                (b_, n_) for b_ in range(1, B_LOCAL)
                for n_ in ("52", "26", "13")]
            for job in work:
                if job is None:
                    # ---------- prologue B: exp -> sqrt eras ----------
                    nc.scalar.activation(wh_t, wh_t, AF.Exp)
                    nc.vector.tensor_mul(
                        wh_t.rearrange("p (b g) k -> p b g k", b=B_LOCAL),
                        wh_t.rearrange("p (b g) k -> p b g k", b=B_LOCAL),
                        ctb[:, _AWV:_AWV + 2 * GTOT].rearrange(
                            "p (g k) -> p g k", k=2).unsqueeze(1)
                        .broadcast_to((128, B_LOCAL, GTOT, 2)))
                    sq_t = single.tile([128, B_LOCAL * GTOT, 2], bf16)
                    nc.vector.tensor_mul(sq_t[:], wh_t, wh_t)
                    q_t = single.tile([128, B_LOCAL * GTOT], bf16)
                    nc.vector.tensor_add(q_t[:], sq_t[:, :, 0], sq_t[:, :, 1])
                    s_t = single.tile([128, B_LOCAL * GTOT], bf16)
                    nc.scalar.activation(s_t[:], q_t[:], AF.Sqrt,
                                         scale=1.0 / (416.0 * 416.0))
                    # pair-duplicated broadcast source for the s pass
                    sm2 = single.tile([128, B_LOCAL * GTOT, 2], bf16)
                    nc.vector.tensor_copy(sm2[:, :, 0], s_t[:])
                    nc.vector.tensor_copy(sm2[:, :, 1], s_t[:])
                else:
                    b, name = job
                    W, t, HW = SCJOB[name]
                    in_t = in_ts[b]
                    T = CHUNKS[name]
                    G = GS[name]
                    go = b * GTOT + GOFF[name]
                    vs = in_t[:, OFFS[name]:OFFS[name] + 3 * HW].rearrange(
                        "p (a hw) -> p a hw", a=3)
                    out_t = out_pool.tile([128, G, 90], bf16, tag="outtile")
                    og = out_t[:]
                    ixyo = _IXY[name]

                    # -- transpose + PSUM evacuation (sigmoid era) --
                    for c0 in range(0, T, 4):
                        nch = min(4, T - c0)
                        gcg = nch * 3
                        ps = psum_pool.tile([128, 12, 128], bf16, tag="ps")
                        for ci in range(nch):
                            c = c0 + ci
                            w = min(128, HW - c * 128)
                            for a in range(3):
                                nc.tensor.transpose(
                                    ps[:w, ci * 3 + a, 0:90],
                                    vs[:, a, c * 128:c * 128 + w],
                                    ident[0:90, 0:90])
                        pg = ps[:, 0:gcg, :]
                        osl = og[:, c0 * 3:c0 * 3 + gcg, :]
                        # seg sigmoids -> contiguous cols 42:90
                        nc.scalar.activation(
                            osl[:, :, 42:90].rearrange(
                                "p g (i j) -> p g i j", j=2),
                            pg[:, :, 18:90].rearrange(
                                "p g (i j) -> p g i j", j=3)[:, :, :, 1:3],
                            AF.Sigmoid)
                        # dx,dy + grid -> cols 2:4 (fused from PSUM)
                        _ixy_ins = nc.vector.tensor_add(
                            osl.rearrange("p (c a) k -> p c a k", a=3)[
                                :, :, :, 2:4],
                            pg.rearrange("p (c a) k -> p c a k", a=3)[
                                :, :, :, 1:3],
                            ctb[:, ixyo + 2 * c0:ixyo + 2 * (c0 + nch)]
                            .rearrange("p (c k) -> p c k", k=2).unsqueeze(2)
                            .broadcast_to((128, nch, 3, 2)))
                        if prev_fin is not None:
                            # schedule hint: finish image b-1's finishers
                            # before image b's DVE evac work, so stores
                            # flow early instead of piling into a
                            # terminal drain
                            add_dep_helper(_ixy_ins.ins, prev_fin.ins,
                                           sync=True,
                                           reason="drain stores early")
                            prev_fin = None
                        # raw point block -> cols 6:18 (alternate engines
                        # to balance ACT/DVE in the evacuation path)
                        if (c0 // 4) % 2 == 0:
                            nc.scalar.copy(osl[:, :, 6:18], pg[:, :, 6:18])
                        else:
                            nc.vector.tensor_copy(osl[:, :, 6:18],
                                                  pg[:, :, 6:18])
                        # raw seg coords -> contiguous cols 18:42
                        nc.vector.tensor_copy(osl[:, :, 18:42],
                                              pg[:, :, 18:90:3])
                    fin_q.append((b, name, t, go, G, out_t))

                # finishers run once sm2 exists (deferred for the two
                # pre-prologue scales)
                if sm2 is None:
                    continue
                for fb, fname, ft, fgo, fG, fout_t in fin_q:
                    fog = fout_t[:]
                    halves = ([(0, fG // 2), (fG // 2, fG)]
                              if fname == "52" else [(0, fG)])
                    for g0, g1 in halves:
                        gl = g1 - g0
                        ogh = fog[:, g0:g1, :]
                        gh = fgo + g0
                        # [mask, sig_raw] pair from prologue
                        nc.vector.tensor_copy(
                            ogh[:, :, 0:2], nm01[:, gh:gh + gl, :])
                        # w,h raw copy from prologue
                        nc.vector.tensor_copy(ogh[:, :, 4:6],
                                              wh_t[:, gh:gh + gl, :])
                        # cx,cy: * t via ACT input-scale (offloads DVE)
                        nc.scalar.activation(
                            ogh[:, :, 2:4], ogh[:, :, 2:4],
                            AF.Copy, scale=float(ft))
                        # point+seg coords * s (mask applied by the host)
                        last_fin = nc.vector.tensor_mul(
                            ogh[:, :, 6:42].rearrange(
                                "p g (i j) -> p g i j", j=2),
                            ogh[:, :, 6:42].rearrange(
                                "p g (i j) -> p g i j", j=2),
                            sm2[:, gh:gh + gl, :].unsqueeze(2).broadcast_to(
                                (128, gl, 18, 2)))
                        nc.gpsimd.dma_start(
                            out=ys[fname][fb][:, g0 * 90:g1 * 90],
                            in_=ogh.rearrange("p g k -> p (g k)"))
                    if fname == ("52" if fb == 0 else "13"):
                        prev_fin = last_fin
                fin_q = []
    nc.compile()
    return nc


def _host_consts(core, anchors, thresh):
    ctf = np.zeros((128, NCF), f32np)
    ctf[:, _THR] = f32np(thresh[0])
    for b in range(B_LOCAL):
        ctf[:, _NTAB + b] = f32np(core * B_LOCAL + b)
    ctb = np.zeros((128, NCB), bf16np)
    # anchor vector per og column group g=(scale,chunk,anchor), interleaved
    awv = np.zeros((GTOT, 2), f32np)
    for name, W, t, HW in SCALES:
        a = anchors[name].astype(f32np)  # [3, 2]
        go = GOFF[name]
        T = CHUNKS[name]
        awv[go:go + GS[name]] = np.tile(a, (T, 1))
        hw = np.arange(T)[None, :] * 128 + np.arange(128)[:, None]  # [128, T]
        o = _IXY[name]
        ctb[:, o:o + 2 * T:2] = (hw % W).astype(bf16np)
        ctb[:, o + 1:o + 2 * T:2] = (hw // W).astype(bf16np)
    ctb[:, _AWV:_AWV + 2 * GTOT] = awv.reshape(-1)[None, :].astype(bf16np)
    return ctf, ctb


def _f32_to_bf16(x):
    """Round-to-nearest-even fp32 -> bf16, much faster than ml_dtypes astype."""
    u = np.ascontiguousarray(x, f32np).view(np.uint32)
    r = ((u + np.uint32(0x7FFF) + ((u >> np.uint32(16)) & np.uint32(1)))
         >> np.uint32(16)).astype(np.uint16)
    return r.view(bf16np)


def _bf16_to_f32(x):
    u = np.asarray(x).view(np.uint16).astype(np.uint32) << np.uint32(16)
    return u.view(f32np)


def _make_in_maps(out13, out26, out52, anchors, thresh):
    xs_full = {
        "13": np.asarray(out13, f32np).reshape(B, 3, 90, 169),
        "26": np.asarray(out26, f32np).reshape(B, 3, 90, 676),
        "52": np.asarray(out52, f32np).reshape(B, 3, 90, 2704),
    }
    # pack per image: [90, concat over scales of (a, hw)] in bf16
    xin = np.empty((B, 90, XIN_F), bf16np)
    # host-pre-transposed planes: pin (p, fp32), whin (dw,dh, bf16)
    pin = np.zeros((B, 128, GTOT), f32np)
    whin = np.zeros((B, 128, GTOT, 2), bf16np)
    for name, _, _, HW in SCALES:
        o = OFFS[name]
        xin[:, :, o:o + 3 * HW] = _f32_to_bf16(xs_full[name].transpose(
            0, 2, 1, 3)).reshape(B, 90, 3 * HW)
        T = CHUNKS[name]
        go = GOFF[name]
        p = np.zeros((B, 3, T * 128), f32np)
        p[:, :, :HW] = xs_full[name][:, :, 0, :]
        pin[:, :, go:go + GS[name]] = p.reshape(B, 3, T, 128).transpose(
            0, 3, 2, 1).reshape(B, 128, T * 3)
        wh = np.zeros((B, 3, 2, T * 128), f32np)
        wh[:, :, :, :HW] = xs_full[name][:, :, 3:5, :]
        whin[:, :, go:go + GS[name], :] = _f32_to_bf16(
            wh.reshape(B, 3, 2, T, 128).transpose(0, 4, 3, 1, 2)).reshape(
                B, 128, T * 3, 2)
    in_maps = []
    for core in range(N_CORES):
        bs = slice(core * B_LOCAL, (core + 1) * B_LOCAL)
        ctf, ctb = _host_consts(core, anchors, thresh)
        cfp = np.concatenate(
            [ctf, pin[bs].transpose(1, 0, 2).reshape(128, B_LOCAL * GTOT)],
            axis=1)
        cwh = np.concatenate(
            [ctb, whin[bs].transpose(1, 0, 2, 3).reshape(
                128, B_LOCAL * GTOT * 2)], axis=1)
        m = {"xin": np.ascontiguousarray(xin[bs]),
             "cfp": np.ascontiguousarray(cfp),
             "cwh": np.ascontiguousarray(cwh)}
        in_maps.append(m)
    return in_maps


def kernel(out13, out26, out52, anchors13, anchors26, anchors52, thresh,
           case, **kw):
    from concourse.bass_utils import run_bass_kernel_spmd

    anchors = {"13": np.asarray(anchors13), "26": np.asarray(anchors26),
               "52": np.asarray(anchors52)}
    thresh = np.asarray(thresh, f32np)

    if "nc" not in _CACHE:
        _CACHE["nc"] = _build_nc()
    nc = _CACHE["nc"]

    in_maps = _make_in_maps(out13, out26, out52, anchors, thresh)
    res = run_bass_kernel_spmd(nc, in_maps, list(range(N_CORES))).results

    rows = {name: B * HW * 3 for name, _, _, HW in SCALES}
    out = np.empty((rows["13"] + rows["26"] + rows["52"], 90), f32np)
    region = {"13": 0, "26": rows["13"], "52": rows["13"] + rows["26"]}
    colperm = np.asarray(COLPERM)
    for core in range(N_CORES):
        r = res[core]
        for name, _, _, HW in SCALES:
            T = CHUNKS[name]
            # [B_LOCAL, 128, T, 3, 90] -> [B_LOCAL, T, 128, 3, 90] -> rows,
            # un-permuting columns and upconverting in the same pass; the
            # device ships raw rows with the exact 0/1 mask in col 0 — the
            # host applies the row mask and fills in n = image index
            arr = _bf16_to_f32(np.asarray(r[f"y{name}"])).reshape(
                B_LOCAL, 128, T, 3, 90).transpose(0, 2, 1, 3, 4)
            arr = arr[..., colperm].reshape(
                B_LOCAL, T * 128, 3, 90)[:, :HW]
            m = arr[..., 0:1]
            arr = arr * m
            nvec = (core * B_LOCAL + np.arange(B_LOCAL)).astype(f32np)
            arr[..., 0] = m[..., 0] * nvec[:, None, None]
            n = B_LOCAL * HW * 3
            out[region[name] + core * n:region[name] + (core + 1) * n] = \
                arr.reshape(n, 90)
    return out
